# revision 7
# baseline (speedup 1.0000x reference)
"""ColorGNN Trainium2 kernel: 3-layer message passing on the complete bipartite
graph (50000 birds x 16 colors, H=128), sharded by birds across 8 NeuronCores.

Reformulation (validated vs reference):
  Split eW1[l] into W1a (bird-x part), W1b (color-x part), W1c (edge-attr part).
  Track h^l = relu(edge_in @ eW1[l] + eb1[l]) instead of edge_attr: then
    h^0      = relu(p * u0 + A0[i] + B0[c] + c0),   u0 = edge_W @ W1c0
    h^l      = relu(h^{l-1} @ Wf_l + A_l[i] + B_l[c] + c_l),  Wf_l = W2_{l-1} @ W1c_l
    aggr     = (sum h) @ W2_l + deg*eb2_l   (aggregation commutes with W2)
  so the only per-edge matmuls are the layer transitions; the scatter-adds
  become per-bird sums (strided PE matmuls / DVE trees) + per-color sums
  (DVE add-tree, AllReduce of [H,16]).

Layout: per-tile h kept H-major [H=128 partitions, edges free], bird-major edge
order (e = i*16 + c), TB=96 birds per tile so ne=1536 = exactly 3 PSUM banks:
one wide psum tile per h stage -> a single ACT relu call. Two-hot lhsT rows:
birds 0..95 (A), colors 96..111 (B), row 112 = u0 (paired with per-tile p row
for layer 0). B lands at psum partitions 96..111 via the matmul's natural
tile_position, so refreshing ab slots per layer is a lane-aligned copy (no DMA).
Final scores are returned without +color_b / *probs; the host applies both.
"""

import numpy as np
import ml_dtypes

import concourse.bass as bass
import concourse.mybir as mybir
import concourse.tile as tile
from concourse.bass_utils import run_bass_kernel_spmd

F32 = mybir.dt.float32
BF16 = mybir.dt.bfloat16
AF = mybir.ActivationFunctionType

NCORES = 8
N, C, H, L = 50000, 16, 128, 3
NB = N // NCORES            # 6250 birds per core
TB = 96                     # birds per tile (96 + 16 colors + 1 u0 = 113 rows)
NT = (NB + TB - 1) // TB    # 66 tiles (last has 10 birds)
NE = TB * C                 # 1536 edge columns per full tile = 3 PSUM banks
CROW = TB                   # first color row in the two-hot lhsT
UROW = TB + C               # u0 / p row


def _split_multi_waits(nc):
    """walrus in this env allows only ONE sync-wait per instruction. For any
    instruction with more waits, hoist the extras onto same-engine nops
    inserted immediately before it (sequencers execute in program order)."""
    k = 0
    for f in nc.m.functions:
        for blk in f.blocks:
            insts = blk.instructions
            out = []
            for inst in insts:
                si = inst.sync_info
                if si is not None and si.on_wait and len(si.on_wait) > 1:
                    waits = list(si.on_wait)
                    for w in waits[:-1]:
                        nop = mybir.InstNoOp(
                            name=f"waitnop-{k}", engine=inst.engine
                        )
                        k += 1
                        nop.sync_info = mybir.SyncInfo(on_wait=[w], on_update=[])
                        out.append(nop)
                    si.on_wait = waits[-1:]
                out.append(inst)
            if len(out) != len(insts):
                blk.instructions = out


def _bf(a):
    return np.ascontiguousarray(np.asarray(a, np.float64)).astype(ml_dtypes.bfloat16)


def _f32(a):
    return np.ascontiguousarray(np.asarray(a, np.float64).astype(np.float32))


def _consts(inp):
    """Host-side weight folding. Returns dict of name -> np array (replicated)."""
    f = {k: np.asarray(v, np.float64) for k, v in inp.items()}
    eW1, eb1, eW2, eb2 = f["eW1"], f["eb1"], f["eW2"], f["eb2"]
    nW1, nb1, nW2, nb2 = f["nW1"], f["nb1"], f["nW2"], f["nb2"]
    W1a = [eW1[l][:H] for l in range(L)]
    W1b = [eW1[l][H : 2 * H] for l in range(L)]
    W1c = [eW1[l][2 * H :] for l in range(L)]
    W2 = [eW2[l] for l in range(L)]
    U1 = [nW1[l][:H] for l in range(L)]
    U2 = [nW1[l][H:] for l in range(L)]

    c = {}
    u0 = f["edge_W"][0] @ W1c[0]                # [H]
    cvec = [
        f["edge_b"] @ W1c[0] + eb1[0],
        eb2[0] @ W1c[1] + eb1[1],
        eb2[1] @ W1c[2] + eb1[2],
    ]
    wmats, vvecs = {}, {}
    for l in range(L):
        vvecs[f"cvec{l}"] = cvec[l]
        wmats[f"W1a{l}"] = W1a[l]               # rhs [H, H]
        wmats[f"W1b{l}"] = W1b[l]               # rhs [H, H]
        wmats[f"U1{l}"] = U1[l]                 # lhsT [H, H]
        wmats[f"W2U2{l}"] = W2[l] @ U2[l]       # lhsT [H, H]
        wmats[f"V{l}"] = nW2[l]                 # lhsT [H, H]
        vvecs[f"bb{l}"] = nb1[l] + C * (eb2[l] @ U2[l])
        vvecs[f"nb2{l}"] = nb2[l]
        if l < L - 1:
            vvecs[f"bc{l}"] = nb1[l] + N * (eb2[l] @ U2[l])
    wmats["Wf1"] = W2[0] @ W1c[1]               # lhsT [H, H]
    wmats["Wf2"] = W2[1] @ W1c[2]               # lhsT [H, H]
    # layer-0 folds: x0 = p @ node_W + node_b never materialized on device
    vvecs["cvec0"] = vvecs["cvec0"] + f["node_b"] @ W1a[0]
    vvecs["bb0"] = vvecs["bb0"] + f["node_b"] @ U1[0]
    c["NW1a"] = _bf(f["node_W"] @ W1a[0])       # [C, H]
    c["NWU1"] = _bf(f["node_W"] @ U1[0])        # [C, H]
    c["wpack"] = _bf(np.concatenate([wmats[k] for k in sorted(wmats)], axis=1))
    c["vpack"] = _f32(np.stack([vvecs[k] for k in sorted(vvecs)], axis=1))
    xc0 = np.eye(C) @ f["node_W"] + f["node_b"]     # [C, H] color state l0
    c["xc0T"] = _bf(xc0.T)                           # [H, C]
    c["color_W"] = _bf(f["color_W"])                 # lhsT [H, C]
    # ab slot init: rows CROW..CROW+16 = B0 = xc0 @ W1b0; row UROW = u0;
    # rows UROW+1..127 zero (keeps junk out of the two-hot matmul)
    abi = np.zeros((128 - CROW, H), np.float64)
    abi[:C] = xc0 @ W1b[0]
    abi[C] = u0
    c["ab_init"] = _bf(abi)                          # [32, H]
    # two-hot base: col e -> 1 at row e//16 (bird) and row CROW + e%16
    # (color); row UROW is zero here (l0 writes per-tile p data into it)
    oh = np.zeros((H, NE), np.float32)
    e = np.arange(NE)
    oh[e // C, e] = 1.0
    oh[CROW + (e % C), e] = 1.0
    c["twohot"] = oh.astype(ml_dtypes.bfloat16)
    return c


def build_nc(sim_mode=False, skew=4, hin_bufs=3, hout_bufs=3, h0r=32,
             tree_bufs=2, l1_pe_colors=8):
    nc = bass.Bass(num_devices=1 if sim_mode else NCORES)

    wnames = sorted(
        [f"{nm}{l}" for l in range(L) for nm in ("W1a", "W1b", "U1", "W2U2", "V")]
        + ["Wf1", "Wf2"]
    )
    vnames = sorted(
        [f"cvec{l}" for l in range(L)] + [f"bb{l}" for l in range(L)]
        + [f"nb2{l}" for l in range(L)] + ["bc0", "bc1"]
    )
    cshapes = {
        "xc0T": ([H, C], BF16), "color_W": ([H, C], BF16),
        "twohot": ([H, NE], BF16),
        "NW1a": ([C, H], BF16), "NWU1": ([C, H], BF16),
        "ab_init": ([128 - CROW, H], BF16),
        "wpack": ([H, len(wnames) * H], BF16),
        "vpack": ([H, len(vnames)], F32),
    }

    dins = {}
    for name, (shp, dt) in cshapes.items():
        dins[name] = nc.declare_dram_parameter(name, shp, dt, isOutput=False)
    dins["pTbf"] = nc.declare_dram_parameter("pTbf", [C, NB], BF16, isOutput=False)
    dins["pbf"] = nc.declare_dram_parameter("pbf", [1, NB * C], BF16, isOutput=False)
    out_d = nc.declare_dram_parameter("outT", [C, NB], F32, isOutput=True)

    with tile.TileContext(nc) as tc:
        with (
            tc.tile_pool(name="const", bufs=1) as constp,
            tc.tile_pool(name="xpool", bufs=1) as xpool,
            tc.tile_pool(name="hsb", bufs=3) as hsbp,
            tc.tile_pool(name="tree", bufs=tree_bufs) as treep,
            tc.tile_pool(name="agg", bufs=4) as aggp,
            tc.tile_pool(name="small", bufs=4) as smallp,
            tc.tile_pool(name="hps", bufs=2, space="PSUM") as hps,
            tc.tile_pool(name="sps", bufs=1, space="PSUM") as sps,
            tc.tile_pool(name="dram", bufs=1, space="DRAM") as dramp,
        ):
            # ---- load constants ----
            cs = {}
            for name, (shp, dt) in cshapes.items():
                cs[name] = constp.tile(shp, dt, name=f"c_{name}")
                nc.sync.dma_start(out=cs[name][:], in_=dins[name][:])
            for i, nm in enumerate(wnames):
                cs[nm] = cs["wpack"][:, i * H : (i + 1) * H]
            for i, nm in enumerate(vnames):
                cs[nm] = cs["vpack"][:, i : i + 1]
            pTbf_sb = constp.tile([C, NB], BF16, name="pTbf_sb")
            nc.sync.dma_start(out=pTbf_sb[:], in_=dins["pTbf"][:])
            outT_sb = constp.tile([C, NB], F32, name="outT_sb")

            xT = xpool.tile([H, NT * TB], BF16)       # bird states, H-major
            xcT = xpool.tile([H, C], BF16)            # color states
            nc.sync.dma_start(out=xcT[:], in_=dins["xc0T"][:])
            csum_acc = xpool.tile([H, C], F32)
            csum_acc2 = xpool.tile([H, C], F32)

            H0R = h0r  # h0 tiles resident in SBUF (no DRAM round-trip);
            # l1's h reuses slice t-2 once l1 has consumed it.
            h0_sb = constp.tile([H, H0R * NE], BF16, name="h0_sb")
            h_d = [dramp.tile([H, NT * NE], BF16, name=f"h_d{i}") for i in range(2)]
            AB_SLOTS = 4
            ab_tiles = [xpool.tile([128, H], BF16, name=f"abslot{i}")
                        for i in range(AB_SLOTS)]
            for abt in ab_tiles:   # B0 + u0 + zero tail, constant until l1
                nc.sync.dma_start(out=abt[CROW:, :], in_=dins["ab_init"][:])
            P3H_SLOTS = 3
            p3h = [xpool.tile([H, NE], BF16, name=f"p3h{i}")
                   for i in range(P3H_SLOTS)]
            for ph3 in p3h:        # two-hot base; row UROW gets p per tile
                nc.sync.dma_start(out=ph3[:], in_=dins["twohot"][:])
            cc_in = [dramp.tile([H, C], F32, name=f"cc_in{i}") for i in range(2)]
            cc_out = [dramp.tile([H, C], F32, name=f"cc_out{i}") for i in range(2)]

            def tcols(t):
                nb = min(TB, NB - t * TB)
                return t * TB, nb

            # ================= pass l =================
            for l in range(L):
                last = l == L - 1
                if not last:
                    nc.vector.memset(csum_acc[:], 0.0)
                    nc.vector.memset(csum_acc2[:], 0.0)

                # resident-slice index in h0_sb for (layer, tile), or None
                def resid(t):
                    if l == 0 and t < H0R:
                        return t
                    if l in (1, 2) and 2 <= t < H0R + 2:
                        return t - 2
                    return None

                dbl_cur = [None]  # current double-width pool tile (even t)

                def csum_tree(region, width, acc):
                    """acc += per-color sums over `region` ([H, width], width a
                    multiple of 16) via a bf16 halving tree on DVE."""
                    s1 = treep.tile([H, NE], BF16, tag="tr1")
                    s2 = treep.tile([H, NE // 2], BF16, tag="tr2")
                    cur, w = region, width
                    bufs_cycle = [s1, s2]
                    bi = 0
                    while w > 48:
                        h2 = (w // 32) * 16   # halve, keep 16-col alignment
                        dst = bufs_cycle[bi % 2]
                        bi += 1
                        nc.vector.tensor_add(dst[:, :h2], cur[:, : w - h2],
                                             cur[:, w - h2 : w])
                        cur, w = dst, h2
                    # w in {16, 32, 48}
                    if w == 48:
                        dst = bufs_cycle[bi % 2]
                        nc.vector.tensor_add(dst[:, :16], cur[:, :16],
                                             cur[:, 16:32])
                        nc.vector.tensor_add(dst[:, 16:32], dst[:, :16],
                                             cur[:, 32:48])
                        cur, w = dst[:, 16:32], 16
                    elif w == 32:
                        dst = bufs_cycle[bi % 2]
                        nc.vector.tensor_add(dst[:, :16], cur[:, :16],
                                             cur[:, 16:32])
                        cur = dst[:, :16]
                    else:
                        cur = cur[:, :16]
                    nc.vector.tensor_add(acc[:], acc[:], cur)

                def stage_h(t):
                    """DMA in, A matmul + ab refresh, wide psum h matmuls,
                    one relu, store h, paired csum tree."""
                    t0, nb = tcols(t)
                    ne = nb * C
                    ec0 = t * NE

                    hin = None
                    if l == 0:
                        rh = p3h[t % P3H_SLOTS]
                        nc.sync.dma_start(out=rh[UROW : UROW + 1, :ne],
                                          in_=dins["pbf"][:, t0 * C : t0 * C + ne])
                    elif l == 1 and t < H0R:
                        hin = h0_sb[:, t * NE : t * NE + NE]
                    elif l == 2 and resid(t) is not None:
                        hin = h0_sb[:, resid(t) * NE : resid(t) * NE + NE]
                    else:
                        hin = hsbp.tile([H, NE], BF16, tag="hin", name="hin",
                                        bufs=hin_bufs)
                        nc.sync.dma_start(out=hin[:, :ne],
                                          in_=h_d[l - 1][:, ec0 : ec0 + ne])

                    # A_l = x^l @ W1a_l, bird-major [nb, H], rows 0..95 of the
                    # apsb psum tile (B_l sits at rows 96..111 via tail matmul)
                    apsb = sps.tile([128, H], F32, name="apsb", tag="apsb")
                    if l == 0:
                        nc.tensor.matmul(apsb[:nb, :],
                                         lhsT=pTbf_sb[:, t0 : t0 + nb],
                                         rhs=cs["NW1a"][:], start=True, stop=True)
                    else:
                        nc.tensor.matmul(apsb[:nb, :], lhsT=xT[:, t0 : t0 + nb],
                                         rhs=cs[f"W1a{l}"][:], start=True, stop=True)
                    ab = ab_tiles[t % AB_SLOTS]
                    if nb < TB:
                        nc.vector.memset(ab[:TB, :], 0.0)
                    if l == 1:
                        nc.scalar.copy(ab[:nb, :], apsb[:nb, :])
                    else:
                        nc.vector.tensor_copy(ab[:nb, :], apsb[:nb, :])

                    # ---- h tile: one wide psum (3 banks), one relu ----
                    wide = hps.tile([H, NE], F32, name="wide")
                    for q0 in range(0, ne, 512):
                        qw = min(512, ne - q0)
                        sl = slice(q0, q0 + qw)
                        if l == 0:
                            nc.tensor.matmul(wide[:, sl], lhsT=ab[:],
                                             rhs=rh[:, sl], start=True, stop=True)
                        else:
                            nc.tensor.matmul(wide[:, sl], lhsT=cs[f"Wf{l}"][:],
                                             rhs=hin[:, sl], start=True, stop=False)
                            nc.tensor.matmul(wide[:, sl], lhsT=ab[:],
                                             rhs=cs["twohot"][:, sl],
                                             start=False, stop=True)
                    rt = resid(t) if l < 2 else None
                    pair_region = None
                    if rt is not None:
                        h_sb = h0_sb[:, rt * NE : rt * NE + NE]
                        if t % 2 == 1:
                            pair_region = h0_sb[:, (rt - 1) * NE : (rt + 1) * NE]
                    elif l == 2:
                        h_sb = hsbp.tile([H, NE], BF16, tag="hout", name="h_sb",
                                         bufs=hout_bufs)
                    else:
                        if t % 2 == 0:
                            dbl_cur[0] = hsbp.tile([H, 2 * NE], BF16, tag="hout",
                                                   name="h_dbl", bufs=hout_bufs)
                            h_sb = dbl_cur[0][:, :NE]
                        else:
                            h_sb = dbl_cur[0][:, NE : NE + ne]
                            pair_region = dbl_cur[0][:, : NE + ne]
                    nc.scalar.activation(h_sb[:, :ne], wide[:, :ne], AF.Relu,
                                         bias=cs[f"cvec{l}"][:])

                    if not last and rt is None:
                        # batched store: both halves of the double on odd t
                        if t % 2 == 1:
                            nc.sync.dma_start(
                                out=h_d[l][:, ec0 - NE : ec0 + ne],
                                in_=dbl_cur[0][:, : NE + ne])
                        elif t == NT - 1:
                            nc.sync.dma_start(out=h_d[l][:, ec0 : ec0 + ne],
                                              in_=h_sb[:, :ne])
                    if not last:
                        acc = csum_acc if t % 2 == 0 else csum_acc2
                        if t % 2 == 1 and pair_region is not None:
                            csum_tree(pair_region, NE + ne, acc)
                        elif t % 2 == 1 or t == NT - 1:
                            csum_tree(h_sb, ne, acc)
                            if t % 2 == 1:
                                # paired emission skipped: do the even tile too
                                pass
                    return h_sb

                def stage_node(t, h_sb):
                    """Bird node update for tile t (+ scores on the last pass).
                    l<2: the bird-sum over each bird's 16 edges rides the z
                    matmul group as 16 strided accumulating matmuls. l==2: DVE
                    color-halving tree + one projection matmul (frees PE)."""
                    t0, nb = tcols(t)
                    ne = nb * C
                    h3 = h_sb[:, :ne].rearrange("p (b c) -> p b c", c=C)
                    node = sps.tile([H, 512], F32, name="node", tag="node")
                    z_ps = node[:, 0:TB]
                    if l == 0:
                        nc.tensor.matmul(z_ps[:, :nb], lhsT=cs["NWU1"][:],
                                         rhs=pTbf_sb[:, t0 : t0 + nb],
                                         start=True, stop=False)
                    else:
                        nc.tensor.matmul(z_ps[:, :nb], lhsT=cs[f"U1{l}"][:],
                                         rhs=xT[:, t0 : t0 + nb],
                                         start=True, stop=False)
                    if last:
                        # bird-sum on DVE: halve colors 16->8->4->2->1
                        b1 = treep.tile([H, NE // 2], BF16, tag="tr1")
                        b2 = treep.tile([H, NE // 4], BF16, tag="tr2")
                        b1v = b1[:, : nb * 8].rearrange("p (b c) -> p b c", c=8)
                        b2v = b2[:, : nb * 4].rearrange("p (b c) -> p b c", c=4)
                        nc.vector.tensor_add(b1v, h3[:, :, 0:8], h3[:, :, 8:16])
                        nc.vector.tensor_add(b2v, b1v[:, :, 0:4], b1v[:, :, 4:8])
                        b3v = b1[:, : nb * 2].rearrange("p (b c) -> p b c", c=2)
                        nc.vector.tensor_add(b3v, b2v[:, :, 0:2], b2v[:, :, 2:4])
                        nc.vector.tensor_add(b2[:, :nb], b3v[:, :, 0],
                                             b3v[:, :, 1])
                        nc.tensor.matmul(z_ps[:, :nb], lhsT=cs[f"W2U2{l}"][:],
                                         rhs=b2[:, :nb], start=False, stop=True)
                    else:
                        for cc in range(C):
                            nc.tensor.matmul(z_ps[:, :nb], lhsT=cs[f"W2U2{l}"][:],
                                             rhs=h3[:, :, cc], start=False,
                                             stop=(cc == C - 1))
                    s_sb = smallp.tile([H, TB], BF16, tag="ssb", name="s_sb")
                    nc.scalar.activation(s_sb[:, :nb], z_ps[:, :nb], AF.Relu,
                                         bias=cs[f"bb{l}"][:])
                    x_ps = node[:, TB : TB + TB]
                    nc.tensor.matmul(x_ps[:, :nb], lhsT=cs[f"V{l}"][:],
                                     rhs=s_sb[:, :nb], start=True, stop=True)
                    nc.vector.tensor_scalar_add(xT[:, t0 : t0 + nb],
                                                x_ps[:, :nb], cs[f"nb2{l}"][:])

                    if last:
                        # scoresT = color_W.T @ x3 (color_b & *p applied on host)
                        sc_ps = node[:C, 2 * TB : 2 * TB + TB]
                        nc.tensor.matmul(sc_ps[:, :nb], lhsT=cs["color_W"][:],
                                         rhs=xT[:, t0 : t0 + nb],
                                         start=True, stop=True)
                        nc.vector.tensor_copy(outT_sb[:, t0 : t0 + nb],
                                              sc_ps[:, :nb])

                # software pipeline: tile t's node update is emitted after
                # tile t+skew's h stage; the layer's collective is kicked off
                # right after the LAST h stage so it overlaps the node tail
                pend = {}
                for t in range(NT + skew):
                    if t < NT:
                        pend[t] = stage_h(t)
                    if t == NT - 1 and not last:
                        nc.vector.tensor_add(csum_acc[:], csum_acc[:],
                                             csum_acc2[:])
                        nc.sync.dma_start(out=cc_in[l][:], in_=csum_acc[:])
                        if sim_mode:
                            nc.sync.dma_start(out=cc_out[l][:], in_=cc_in[l][:])
                        else:
                            nc.gpsimd.collective_compute(
                                "AllReduce", mybir.AluOpType.add,
                                replica_groups=[list(range(NCORES))],
                                ins=[cc_in[l][:].opt()], outs=[cc_out[l][:].opt()],
                            )
                    if t >= skew:
                        stage_node(t - skew, pend.pop(t - skew))
                if last:
                    nc.sync.dma_start(out=out_d[:], in_=outT_sb[:])

                # ---- layer tail: color update + next-layer B (l < 2) ----
                if not last:
                    csg = smallp.tile([H, C], F32, tag="csg")
                    nc.sync.dma_start(out=csg[:], in_=cc_out[l][:])
                    csg_bf = smallp.tile([H, C], BF16, tag="csgbf")
                    nc.vector.tensor_copy(csg_bf[:], csg[:])
                    node = sps.tile([H, 512], F32, name="node", tag="node")
                    zc_ps = node[:, 448 : 448 + C]
                    nc.tensor.matmul(zc_ps[:], lhsT=cs[f"U1{l}"][:], rhs=xcT[:],
                                     start=True, stop=False)
                    nc.tensor.matmul(zc_ps[:], lhsT=cs[f"W2U2{l}"][:],
                                     rhs=csg_bf[:], start=False, stop=True)
                    sc2 = smallp.tile([H, C], BF16, tag="sc2")
                    nc.scalar.activation(sc2[:], zc_ps[:], AF.Relu,
                                         bias=cs[f"bc{l}"][:])
                    xc_ps = node[:, 464 : 464 + C]
                    nc.tensor.matmul(xc_ps[:], lhsT=cs[f"V{l}"][:], rhs=sc2[:],
                                     start=True, stop=True)
                    nc.scalar.activation(xcT[:], xc_ps[:], AF.Identity,
                                         bias=cs[f"nb2{l}"][:])
                    # B_{l+1} = xc^{l+1} @ W1b_{l+1}: lands on psum partitions
                    # 96..111, then a lane-aligned copy into each ab slot
                    apsb = sps.tile([128, H], F32, name="apsb", tag="apsb")
                    nc.tensor.matmul(apsb[CROW : CROW + C, :], lhsT=xcT[:],
                                     rhs=cs[f"W1b{l + 1}"][:],
                                     start=True, stop=True,
                                     tile_position=(0, CROW))
                    for abt in ab_tiles:
                        nc.scalar.copy(abt[CROW : CROW + C, :],
                                       apsb[CROW : CROW + C, :])

    _split_multi_waits(nc)
    return nc


def make_in_maps(inputs):
    c = _consts(inputs)
    probs = np.asarray(inputs["probs"], np.float32)
    in_maps = []
    for k in range(NCORES):
        sl = probs[k * NB : (k + 1) * NB]          # [NB, C]
        m = dict(c)
        m["pTbf"] = np.ascontiguousarray(sl.T).astype(ml_dtypes.bfloat16)
        m["pbf"] = sl.reshape(1, -1).astype(ml_dtypes.bfloat16)  # [1, NB*C]
        in_maps.append(m)
    return in_maps


_NC_CACHE = None


def kernel(**inputs) -> np.ndarray:
    global _NC_CACHE
    if _NC_CACHE is None:
        _NC_CACHE = build_nc()
    nc = _NC_CACHE
    in_maps = make_in_maps(inputs)
    res = run_bass_kernel_spmd(nc, in_maps, core_ids=list(range(NCORES)))
    outT = np.concatenate([res.results[k]["outT"] for k in range(NCORES)], axis=1)
    scores = np.ascontiguousarray(outT.T).astype(np.float32)
    out = (scores + np.asarray(inputs["color_b"], np.float32)) \
        * np.asarray(inputs["probs"], np.float32)
    return out.astype(np.float32)
